# revision 17
# baseline (speedup 1.0000x reference)
"""Trainium2 Bass kernel for the DPPNMT seq2seq LSTM+attention model (v5).

Sharding: data-parallel over batch (64 -> 8 per core, 8 cores), params
replicated. Each core runs encoder+decoder+vocab projection+logsumexp for
its 8 batch elements; host combines per-core gold/lse partials into (64,).

v5 (596.7us on HW, from the 889us v2 baseline):
- LSTM cell via fused scalar_tensor_tensor ops (u=(tanh(f,i)+1)*(c,g),
  h=(tanh(o)+1)*tanh(c)); single-source recurrent consumers halve the
  gate matmul count (encoder 33->17 per step/dir).
- z-tanh split per gate group (g,f,i as soon as their O-part matmuls
  land; o later, off the critical path) in both encoder and decoder.
- Softmax: exp writes straight into the block-diag alpha tile (two
  partition-half ACTs, no copies); Z broadcast to all 128 partitions by
  one all-ones matmul so 1/Z is a full-partition reciprocal and
  a_t = pat * rz is a single tensor_mul.
- Decoder PSUM split into four single-buffer bank tags (z | peB+Z | pat
  | po): tile-granular WAR/RAW tracking no longer serializes exp behind
  gate/Wcomb writes (+the emission order keeps gates(t+1) off exp's
  semaphore target).
- Attention score stationaries stored b-major (epv2) so their
  LDWEIGHTS are contiguous (2x faster issue than the strided layout).
- fp8e4 DoubleRow ONLY for the 512-col vocab-projection streams (K=256
  in one matmul; O rows get a per-step fp8 copy). Measured on HW:
  DoubleRow costs ~127ns/matmul vs ~27ns for small-N bf16 pairs, so the
  gate/score/Wcomb matmuls stay bf16.
- Vocab chunks interleaved into decoder steps t>=16 (4 x 512-col chunks
  + exps per step, placed in the ACT-idle windows of the serial cycle);
  encoder transposes double-buffered through the pep pool.
"""

from contextlib import ExitStack

import numpy as np
import ml_dtypes

import concourse.bass as bass
import concourse.tile as tile
from concourse import bacc, mybir
from concourse.bass_utils import run_bass_kernel_spmd
from concourse.masks import make_identity

BF16 = mybir.dt.bfloat16
F32 = mybir.dt.float32
FP8 = mybir.dt.float8e4
AF = mybir.ActivationFunctionType
ALU = mybir.AluOpType
DR = mybir.MatmulPerfMode.DoubleRow

S, T, B, E, H, V = 64, 64, 64, 256, 256, 32000
NCORES = 8
BL = B // NCORES          # local batch = 8
TD = T - 1                # decoder steps = 63
GCH = 8                   # gate chunks (4H/128)
ECH = 2
HCH = 2
NR = TD * BL              # 504 vocab rows per core
VBLK = 512                # vocab cols per chunk (1 psum bank)
NVS = 63                  # chunks per mt pass (62 full + 1 ragged 256)
WSCL = 64.0               # fp8 Wvocab scale
GSCL = 16.0               # fp8 gate/Wcomb weight scale
ESCL = 8.0                # encproj scale
bf16 = ml_dtypes.bfloat16

HST = (S + 1) * 8
OST = (TD + 1) * 8


def build_program():
    nc = bacc.Bacc("TRN2", target_bir_lowering=False, debug=False)

    def din(name, shape, dt=BF16):
        return nc.dram_tensor(name, shape, dt, kind="ExternalInput").ap()

    xf_t = din("xf_t", [128, ECH * S * BL])
    xb_t = din("xb_t", [128, ECH * S * BL])
    wih_f = din("wih_f", [128, ECH * GCH * 128])
    wih_b = din("wih_b", [128, ECH * GCH * 128])
    whh_f = din("whh_f", [128, HCH * GCH * 128])
    whh_b = din("whh_b", [128, HCH * GCH * 128])
    benc_f = din("benc_f", [128, GCH], F32)
    benc_b = din("benc_b", [128, GCH], F32)
    yt = din("yt", [128, ECH * TD * BL])
    wihe = din("wihe", [128, ECH * GCH * 128])
    wiho = din("wiho", [128, HCH * GCH * 128])
    whhd = din("whhd", [128, HCH * GCH * 128])
    bdec = din("bdec", [128, GCH], F32)
    wcomb_l = din("wcomb_l", [128, 6 * 2 * 128])
    wh_l = din("wh_l", [128, 4 * 2 * 128])
    wc_l = din("wc_l", [128, 4 * 2 * 128])
    watt_l = din("watt_l", [128, 4 * 2 * 128])
    wvt = din("wvt", [128, HCH * V], FP8)
    wgt = din("wgt", [128, HCH * NR])
    out_lse = nc.dram_tensor("out_lse", [128, 4], F32,
                             kind="ExternalOutput").ap()
    out_gd = nc.dram_tensor("out_gd", [1, 1024], F32,
                            kind="ExternalOutput").ap()

    with tile.TileContext(nc) as tc:
        with ExitStack() as ctx:
            consts = ctx.enter_context(tc.tile_pool(name="consts", bufs=1))
            wsb = ctx.enter_context(tc.tile_pool(name="wsb", bufs=1))
            state = ctx.enter_context(tc.tile_pool(name="state", bufs=1))

            id128 = consts.tile([128, 128], BF16)
            make_identity(nc, id128[:])
            ones_bf = consts.tile([128, 1], BF16)
            nc.vector.memset(ones_bf[:], 1.0)
            ones_all = consts.tile([128, 128], BF16)
            nc.vector.memset(ones_all[:], 1.0)

            def load(ap_dram, dt=BF16):
                t = wsb.tile(list(ap_dram.shape), dt,
                             tag=ap_dram.tensor.name + "_sb")
                nc.sync.dma_start(t[:], ap_dram[:])
                return t

            xf_sb, xb_sb = load(xf_t), load(xb_t)
            wihf_sb, wihb_sb = load(wih_f), load(wih_b)
            whhf_sb, whhb_sb = load(whh_f), load(whh_b)
            bencf_sb, bencb_sb = load(benc_f, F32), load(benc_b, F32)
            yt_sb = load(yt)
            wihe_sb = load(wihe)
            wiho_sb, whhd_sb = load(wiho), load(whhd)
            bdec_sb = load(bdec, F32)
            wcomb_sb = load(wcomb_l)
            wh_sb, wc_sb, watt_sb = load(wh_l), load(wc_l), load(watt_l)
            wgt_sb = load(wgt)

            # resident fp8 Wvocab^T, streamed in 8 DMA pieces
            wv_sb = state.tile([128, HCH * V], FP8)
            for p in range(8):
                w0 = p * (HCH * V // 8)
                w1 = (p + 1) * (HCH * V // 8)
                nc.sync.dma_start(wv_sb[:, w0:w1], wvt[:, w0:w1])

            # persistent state
            hf_all = state.tile([128, 2 * HST], BF16)
            hb_all = state.tile([128, 2 * HST], BF16)
            for hx in (hf_all, hb_all):
                nc.vector.memset(hx[:, 0:8], 0.0)
                nc.vector.memset(hx[:, HST:HST + 8], 0.0)
            # cell tiles: [c(0:16) | tg(16:32) | f i o]
            Wf = state.tile([128, 80], F32)
            Wb = state.tile([128, 80], F32)
            Wd = state.tile([128, 80], F32)
            nc.vector.memset(Wf[:, 0:16], 0.0)
            nc.vector.memset(Wb[:, 0:16], 0.0)
            outsT = state.tile([128, 2 * OST], BF16)
            nc.vector.memset(outsT[:, 0:8], 0.0)
            nc.vector.memset(outsT[:, OST:OST + 8], 0.0)
            outsT8 = state.tile([128, 2 * OST], FP8)
            hdec0 = state.tile([128, 16], BF16)
            zxf = state.tile([128, S * 64], BF16)
            zxb = state.tile([128, S * 64], BF16)
            zyb = state.tile([128, TD * 64], BF16)
            ehs_cs = state.tile([128, 16 * 128], BF16)
            encprojT = state.tile([128, HCH * BL * S], BF16)
            epv2 = state.tile([128, BL * HCH * S], BF16)
            ablk = state.tile([128, 8], BF16)
            nc.vector.memset(ablk[:], 0.0)
            se_parts = state.tile([128, 3 * NVS + 16], F32)
            nc.vector.memset(se_parts[:], 1.0)
            lse_sb = state.tile([128, 4], F32)
            gd_sb = state.tile([1, 1024], F32)
            nc.vector.memset(gd_sb[:], 0.0)
            tmp_gd = state.tile([128, 2 * NR], BF16)
            rz_sb = state.tile([128, 8], F32)

            with ExitStack() as rctx:
                pep = rctx.enter_context(
                    tc.tile_pool(name="pep", bufs=2, space="PSUM"))
                pz = rctx.enter_context(
                    tc.tile_pool(name="pz", bufs=2, space="PSUM"))
                psmall = rctx.enter_context(
                    tc.tile_pool(name="psmall", bufs=1, space="PSUM"))
                work = rctx.enter_context(tc.tile_pool(name="work", bufs=2))

                # ---- zx = x @ Wih^T + b; zyb phase deferred past the
                # encoder so its Act work fills encoder idle ----
                def zx_phase(x_sb, wih_sb, b_sb, zx, nt):
                    zxv = zx[:].rearrange("p (t g b) -> p t g b",
                                          g=GCH, b=BL)
                    for gch in range(GCH):
                        ps = pep.tile([128, S * BL], F32, tag="pep",
                                      name="ps")
                        for ech in range(ECH):
                            nc.tensor.matmul(
                                ps[:, 0:nt * BL],
                                wih_sb[:, (ech * GCH + gch) * 128:
                                       (ech * GCH + gch + 1) * 128],
                                x_sb[:, ech * nt * BL:(ech + 1) * nt * BL],
                                start=(ech == 0), stop=(ech == ECH - 1))
                        nc.scalar.activation(
                            zxv[:, 0:nt, gch, :], ps[:, 0:nt * BL],
                            AF.Identity, bias=b_sb[:, gch:gch + 1])

                zx_phase(xf_sb, wihf_sb, bencf_sb, zxf, S)
                zx_phase(xb_sb, wihb_sb, bencb_sb, zxb, S)

                # ---- encoder: two dir chains, op-interleaved emission ----
                hfv = hf_all[:].rearrange("p (c t b) -> p c t b", c=2, b=BL)
                hbv = hb_all[:].rearrange("p (c t b) -> p c t b", c=2, b=BL)

                enc_prev = {}

                def enc_step(Wt, h_all, hv, whh_sb, zx, pfx, t):
                    z = pz.tile([128, 64], F32, tag="z" + pfx, name="z")
                    pt = enc_prev.get(pfx)
                    nc.tensor.matmul(z[:], id128[:],
                                     zx[:, t * 64:(t + 1) * 64],
                                     start=True, stop=(pt is None),
                                     skip_group_check=True)
                    if pt is not None:
                        for gch in range(6):
                            for kch in range(HCH):
                                nc.tensor.matmul(
                                    z[:, gch * 8:(gch + 1) * 8],
                                    whh_sb[:, (kch * GCH + gch) * 128:
                                           (kch * GCH + gch + 1) * 128],
                                    h_all[:, kch * HST + pt * 8:
                                          kch * HST + pt * 8 + 8],
                                    start=False, stop=False,
                                    skip_group_check=True)
                    # g,f,i tanh as soon as their gate chunks are done
                    nc.scalar.activation(Wt[:, 16:64], z[:, 0:48], AF.Tanh)
                    if pt is not None:
                        for gch in (6, 7):
                            for kch in range(HCH):
                                nc.tensor.matmul(
                                    z[:, gch * 8:(gch + 1) * 8],
                                    whh_sb[:, (kch * GCH + gch) * 128:
                                           (kch * GCH + gch + 1) * 128],
                                    h_all[:, kch * HST + pt * 8:
                                          kch * HST + pt * 8 + 8],
                                    start=False,
                                    stop=(gch == GCH - 1 and kch == HCH - 1),
                                    skip_group_check=True)
                    yield
                    nc.scalar.activation(Wt[:, 64:80], z[:, 48:64], AF.Tanh)
                    yield
                    # u = (tanh(f,i)+1) * (c, tg)
                    u = work.tile([128, 32], F32, tag=pfx + "u", name="u")
                    nc.vector.scalar_tensor_tensor(
                        u[:], Wt[:, 32:64], 1.0, Wt[:, 0:32],
                        ALU.add, ALU.mult)
                    yield
                    C2 = work.tile([128, 16], F32, tag=pfx + "C2", name="C2")
                    nc.vector.tensor_add(C2[:], u[:, 0:16], u[:, 16:32])
                    yield
                    tc_ = work.tile([128, 16], BF16, tag=pfx + "tc",
                                    name="tc")
                    nc.scalar.activation(tc_[:], C2[:], AF.Tanh, scale=0.5)
                    yield
                    # h = (tanh(o)+1) * tanh(c')  (single source for gates)
                    ov = Wt[:, 64:80].rearrange("p (c b) -> p c b", c=2)
                    tv = tc_[:].rearrange("p (c b) -> p c b", c=2)
                    nc.vector.scalar_tensor_tensor(
                        hv[:, :, t + 1, :], ov, 1.0, tv,
                        ALU.add, ALU.mult)
                    enc_prev[pfx] = t + 1
                    yield
                    nc.gpsimd.tensor_scalar_mul(Wt[:, 0:16], C2[:], 0.5)

                def drive2(*gens):
                    gens = [gn for gn in gens if gn is not None]
                    while gens:
                        keep = []
                        for gn in gens:
                            try:
                                next(gn)
                                keep.append(gn)
                            except StopIteration:
                                pass
                        gens = keep

                for t in range(S):
                    drive2(
                        enc_step(Wf, hf_all, hfv, whhf_sb, zxf, "f", t),
                        enc_step(Wb, hb_all, hbv, whhb_sb, zxb, "b", t))
                    if t == 2:
                        zx_phase(yt_sb, wihe_sb, bdec_sb, zyb, TD)

                # ---- ehs_cs[64u+s, (pair,mt)] via PE transposes ----
                for pair in range(4):
                    for mt in range(4):
                        srcv = hfv if mt < 2 else hbv
                        pt = pep.tile([128, 128], BF16, tag="pep",
                                      name="pt")
                        for u in range(2):
                            in_ap = srcv[:, mt % 2, 1:S + 1, 2 * pair + u]
                            nc.tensor.transpose(pt[u * 64:(u + 1) * 64, :],
                                                in_ap, id128[:])
                        nc.vector.tensor_copy(
                            ehs_cs[:, (pair * 4 + mt) * 128:
                                   (pair * 4 + mt + 1) * 128], pt[:])

                # ---- encproj^T = (2*Watt) @ ehs^T  (x8 scale for fp8) ----
                for mch in range(HCH):
                    ps = pep.tile([128, S * BL], F32, tag="pep")
                    for kch in range(4):
                        srch = hf_all if kch < 2 else hb_all
                        rhs = srch[:, (kch % 2) * HST + 8:
                                   (kch % 2) * HST + HST]
                        nc.tensor.matmul(
                            ps[:],
                            watt_sb[:, (kch * 2 + mch) * 128:
                                    (kch * 2 + mch + 1) * 128],
                            rhs, start=(kch == 0), stop=(kch == 3))
                    nc.scalar.activation(
                        encprojT[:, mch * BL * S:(mch + 1) * BL * S],
                        ps[:], AF.Copy)
                # b-major copy: per-b score stationaries become contiguous
                e_in = encprojT[:].rearrange("p (c s b) -> p b c s",
                                             c=2, b=BL)
                e_out = epv2[:].rearrange("p (b c s) -> p b c s",
                                          b=BL, c=2)
                nc.vector.tensor_copy(e_out, e_in)

                # ---- decoder init: dec_h/dec_c projections ----
                cfb = work.tile([128, 16], BF16, tag="cfb")
                cbb = work.tile([128, 16], BF16, tag="cbb")
                nc.vector.tensor_copy(cfb[:], Wf[:, 0:16])
                nc.vector.tensor_copy(cbb[:], Wb[:, 0:16])
                pinit = psmall.tile([128, 32], F32, tag="po")
                for (w_sb, off, hsrc, csrc) in (
                        (wh_sb, 0, (hf_all, hb_all), None),
                        (wc_sb, 16, None, (cfb, cbb))):
                    for mch in range(HCH):
                        for kch in range(4):
                            if hsrc is not None:
                                hx = hsrc[0] if kch < 2 else hsrc[1]
                                rhs = hx[:, (kch % 2) * HST + S * 8:
                                         (kch % 2) * HST + S * 8 + 8]
                            else:
                                cx = csrc[0] if kch < 2 else csrc[1]
                                rhs = cx[:, (kch % 2) * 8:(kch % 2) * 8 + 8]
                            nc.tensor.matmul(
                                pinit[:, off + mch * 8:off + (mch + 1) * 8],
                                w_sb[:, (kch * 2 + mch) * 128:
                                     (kch * 2 + mch + 1) * 128],
                                rhs, start=(kch == 0), stop=(kch == 3))
                nc.vector.tensor_copy(hdec0[:], pinit[:, 0:16])
                nc.vector.tensor_copy(Wd[:, 0:16], pinit[:, 16:32])

            # ---- decoder with interleaved vocab chunks ----
            with ExitStack() as dctx:
                pv = dctx.enter_context(
                    tc.tile_pool(name="pv", bufs=4, space="PSUM"))
                pd = dctx.enter_context(
                    tc.tile_pool(name="pd", bufs=1, space="PSUM"))
                workd = dctx.enter_context(tc.tile_pool(name="workd", bufs=2))

                ovv = outsT[:].rearrange("p (c t b) -> p c t b", c=2, b=BL)
                ov8v = outsT8[:].rearrange("p (c t b) -> p c t b", c=2, b=BL)
                ovdr = outsT8[:].rearrange("p (c r) -> p c r", c=2)
                wvdr = wv_sb[:].rearrange("p (c n) -> p c n", c=2)

                # single-buffer PSUM bank tags (decoupled dep tracking)
                z_ps = pd.tile([128, 64], F32, tag="dz")
                sc_ps = pd.tile([128, 12], F32, tag="dsc")
                pat_ps = pd.tile([128, 32], F32, tag="dpat")
                po_ps = pd.tile([128, 16], F32, tag="dpo")
                peB = sc_ps[:, 0:4]
                Zb = sc_ps[:, 4:12]

                chunk_queue = []

                def vocab_mm(mt, vs):
                    w = VBLK if vs < NVS - 1 else (V - (NVS - 1) * VBLK)
                    m = 128 if mt < 3 else 120
                    ntau = 16 if mt < 3 else 15
                    col0 = vs * VBLK
                    pvt = pv.tile([128, VBLK], F32, tag="pv")
                    nc.tensor.matmul(
                        pvt[0:m, 0:w],
                        ovdr[:, :, (mt * 16 + 1) * 8:
                             (mt * 16 + 1 + ntau) * 8],
                        wvdr[:, :, col0:col0 + w],
                        start=True, stop=True, perf_mode=DR)
                    return (mt, vs, pvt, w, m)

                def vocab_exp(rec):
                    mt, vs, pvt, w, m = rec
                    nc.scalar.activation(
                        pvt[0:m, 0:w], pvt[0:m, 0:w], AF.Exp,
                        scale=1.0 / WSCL,
                        accum_out=se_parts[0:m, mt * NVS + vs:
                                           mt * NVS + vs + 1])

                def gates_h(z, h_bf, t_next):
                    """inject + H-part for step t_next."""
                    nc.tensor.matmul(z, id128[:],
                                     zyb[:, t_next * 64:(t_next + 1) * 64],
                                     start=True, stop=False,
                                     skip_group_check=True)
                    for gch in range(GCH):
                        for kch in range(HCH):
                            nc.tensor.matmul(
                                z[:, gch * 8:(gch + 1) * 8],
                                whhd_sb[:, (kch * GCH + gch) * 128:
                                        (kch * GCH + gch + 1) * 128],
                                h_bf[:, kch * 8:(kch + 1) * 8],
                                start=False, stop=False,
                                skip_group_check=True)

                def gates_o(z, t_next, g0, g1):
                    for gch in range(g0, g1):
                        for kch in range(HCH):
                            nc.tensor.matmul(
                                z[:, gch * 8:(gch + 1) * 8],
                                wiho_sb[:, (kch * GCH + gch) * 128:
                                        (kch * GCH + gch + 1) * 128],
                                outsT[:, kch * OST + t_next * 8:
                                      kch * OST + t_next * 8 + 8],
                                start=False,
                                stop=(gch == GCH - 1 and kch == HCH - 1),
                                skip_group_check=True)

                abv = ablk[:].rearrange("p (j u) -> p j u", u=2)

                def tanh1():
                    # g,f,i gate chunks -> Wd[16:64]
                    nc.scalar.activation(Wd[:, 16:64], z_ps[:, 0:48],
                                         AF.Tanh)

                gates_h(z_ps[:], hdec0, 0)
                gates_o(z_ps[:], 0, 0, 6)
                tanh1()
                gates_o(z_ps[:], 0, 6, 8)

                for t in range(TD):
                    if t == 16:
                        chunk_queue.extend((0, vs) for vs in range(NVS))
                    elif t == 32:
                        chunk_queue.extend((1, vs) for vs in range(NVS))
                    elif t == 48:
                        chunk_queue.extend((2, vs) for vs in range(NVS))
                    exps = []
                    if t >= 16:
                        for _ in range(4):
                            if chunk_queue:
                                mt_, vs_ = chunk_queue.pop(0)
                                exps.append(vocab_mm(mt_, vs_))

                    # -- cell (tanh1 of this step ran at end of t-1) --
                    nc.scalar.activation(Wd[:, 64:80], z_ps[:, 48:64],
                                         AF.Tanh)
                    u = workd.tile([128, 32], F32, tag="u")
                    nc.vector.scalar_tensor_tensor(
                        u[:], Wd[:, 32:64], 1.0, Wd[:, 0:32],
                        ALU.add, ALU.mult)
                    C2 = workd.tile([128, 16], F32, tag="C2")
                    nc.vector.tensor_add(C2[:], u[:, 0:16], u[:, 16:32])
                    tc_ = workd.tile([128, 16], BF16, tag="tc")
                    nc.scalar.activation(tc_[:], C2[:], AF.Tanh, scale=0.5)
                    nc.gpsimd.tensor_scalar_mul(Wd[:, 0:16], C2[:], 0.5)
                    hdec = workd.tile([128, 16], BF16, tag="hdec")
                    nc.vector.scalar_tensor_tensor(
                        hdec[:], Wd[:, 64:80], 1.0, tc_[:],
                        ALU.add, ALU.mult)
                    if exps:
                        vocab_exp(exps.pop(0))
                    # -- scores: e = ep . h (contiguous per-b stationaries) --
                    for b in range(BL):
                        u_, j_ = b % 2, b // 2
                        for ch in range(HCH):
                            nc.tensor.matmul(
                                peB[u_ * 64:u_ * 64 + 64, j_:j_ + 1],
                                epv2[:, b * 128 + ch * 64:
                                     b * 128 + ch * 64 + 64],
                                hdec[:, ch * 8 + b:ch * 8 + b + 1],
                                start=(ch == 0), stop=(ch == 1),
                                skip_group_check=True)
                    # -- gates t+1 H-part: fills the exp window --
                    if t + 1 < TD:
                        gates_h(z_ps[:], hdec, t + 1)
                    # -- exp directly into block-diag alpha halves --
                    nc.scalar.activation(abv[0:64, :, 0], peB[0:64, :],
                                         AF.Exp, scale=1.0 / ESCL)
                    nc.scalar.activation(abv[64:128, :, 1], peB[64:128, :],
                                         AF.Exp, scale=1.0 / ESCL)
                    # -- Z broadcast to all partitions: one ones-matmul --
                    nc.tensor.matmul(Zb, ones_all[:], ablk[:, 0:8],
                                     start=True, stop=True,
                                     skip_group_check=True)
                    nc.vector.reciprocal(rz_sb[:], Zb)
                    # -- context (block-diag alpha x ehs) --
                    for pair in range(4):
                        for mt in range(4):
                            nc.tensor.matmul(
                                pat_ps[:, mt * 8 + pair * 2:
                                       mt * 8 + pair * 2 + 2],
                                ehs_cs[:, (pair * 4 + mt) * 128:
                                       (pair * 4 + mt + 1) * 128],
                                ablk[:, 2 * pair:2 * pair + 2],
                                start=True, stop=True,
                                skip_group_check=True)
                    # Wcomb h-part here: runs on PE during recip/aTb
                    # (after pat so it cannot delay Zb/pat behind the exps)
                    for kch in (4, 5):
                        for mch in range(HCH):
                            nc.tensor.matmul(
                                po_ps[:, mch * 8:(mch + 1) * 8],
                                wcomb_sb[:, (kch * 2 + mch) * 128:
                                         (kch * 2 + mch + 1) * 128],
                                hdec[:, (kch - 4) * 8:(kch - 3) * 8],
                                start=(kch == 4), stop=False,
                                skip_group_check=True)
                    if exps:
                        vocab_exp(exps.pop(0))
                    if exps:
                        vocab_exp(exps.pop(0))
                    # -- a_t = pat * (1/Z) --
                    aTb = workd.tile([128, 32], BF16, tag="aTb")
                    pat_v = pat_ps[:].rearrange("p (m b) -> p m b", b=8)
                    rz_v = rz_sb[:].rearrange("p (m b) -> p m b", m=1)
                    pat_b, rz_b = bass.broadcast_tensor_aps(pat_v, rz_v)
                    aTb_v = aTb[:].rearrange("p (m b) -> p m b", b=8)
                    nc.vector.tensor_mul(aTb_v, pat_b, rz_b)
                    # -- Wcomb a-part + output tanh --
                    for kch in range(4):
                        for mch in range(HCH):
                            nc.tensor.matmul(
                                po_ps[:, mch * 8:(mch + 1) * 8],
                                wcomb_sb[:, (kch * 2 + mch) * 128:
                                         (kch * 2 + mch + 1) * 128],
                                aTb[:, kch * 8:(kch + 1) * 8],
                                start=False,
                                stop=(kch == 3 and mch == HCH - 1),
                                skip_group_check=True)
                    nc.scalar.activation(ovv[:, :, t + 1, :],
                                         po_ps[:], AF.Tanh)
                    # fp8 copy of O row for the DoubleRow vocab matmuls
                    nc.vector.tensor_copy(ov8v[:, :, t + 1, :],
                                          ovv[:, :, t + 1, :])
                    if exps:
                        vocab_exp(exps.pop(0))
                    if t + 1 < TD:
                        gates_o(z_ps[:], t + 1, 0, 6)
                        tanh1()
                        gates_o(z_ps[:], t + 1, 6, 8)
                    while exps:
                        vocab_exp(exps.pop(0))

                # ---- non-mt3 leftovers (small chunks) ----
                mm_done = []
                for (mt, vs) in chunk_queue:
                    rec = vocab_mm(mt, vs)
                    mm_done.append(rec)
                    if len(mm_done) > 1:
                        vocab_exp(mm_done.pop(0))
                for rec in mm_done:
                    vocab_exp(rec)

            # ---- mt=3 vocab tail with wide chunks, then gold + lse ----
            with ExitStack() as tctx:
                pvb = tctx.enter_context(
                    tc.tile_pool(name="pvb", bufs=2, space="PSUM"))
                VB2 = 2048
                ovt = outsT8[:].rearrange("p (c r) -> p c r", c=2)
                wvt2 = wv_sb[:].rearrange("p (c n) -> p c n", c=2)
                tail_recs = []
                for vs in range(16):
                    w = VB2 if vs < 15 else (V - 15 * VB2)
                    col0 = vs * VB2
                    pvt = pvb.tile([128, VB2], F32, tag="pvb")
                    for n0 in range(0, w, 512):
                        nw = min(512, w - n0)
                        nc.tensor.matmul(
                            pvt[0:120, n0:n0 + nw],
                            ovt[:, :, 49 * 8:64 * 8],
                            wvt2[:, :, col0 + n0:col0 + n0 + nw],
                            start=True, stop=True, perf_mode=DR)
                    tail_recs.append((vs, pvt, w))
                    if len(tail_recs) > 1:
                        vs_, pvt_, w_ = tail_recs.pop(0)
                        nc.scalar.activation(
                            pvt_[0:120, 0:w_], pvt_[0:120, 0:w_], AF.Exp,
                            scale=1.0 / WSCL,
                            accum_out=se_parts[0:120, 3 * NVS + vs_:
                                               3 * NVS + vs_ + 1])
                for (vs_, pvt_, w_) in tail_recs:
                    nc.scalar.activation(
                        pvt_[0:120, 0:w_], pvt_[0:120, 0:w_], AF.Exp,
                        scale=1.0 / WSCL,
                        accum_out=se_parts[0:120, 3 * NVS + vs_:
                                           3 * NVS + vs_ + 1])

                # gold logits: dot(O_t, Wvocab[gold]) via ones-matmul
                ovf = outsT[:].rearrange("p (c t b) -> p c t b", c=2, b=BL)
                ov = ovf[:, :, 1:, :]
                wgv = wgt_sb[:].rearrange("p (c t b) -> p c t b", c=2, b=BL)
                tgv = tmp_gd[:].rearrange("p (c t b) -> p c t b", c=2, b=BL)
                nc.vector.tensor_mul(tgv, ov, wgv)
                pgd = pvb.tile([128, VB2], F32, tag="pvb", name="pgd")
                nc.tensor.matmul(pgd[0:1, 0:NR], ones_bf[:],
                                 tmp_gd[:, 0:NR], start=True, stop=True)
                nc.tensor.matmul(pgd[0:1, 512:512 + NR], ones_bf[:],
                                 tmp_gd[:, NR:2 * NR], start=True, stop=True)
                nc.scalar.activation(gd_sb[:, 0:NR], pgd[0:1, 0:NR], AF.Copy)
                nc.scalar.activation(gd_sb[:, 512:512 + NR],
                                     pgd[0:1, 512:512 + NR], AF.Copy)

                # lse reduce + ln
                for mt in range(3):
                    nc.vector.tensor_reduce(
                        lse_sb[:, mt:mt + 1],
                        se_parts[:, mt * NVS:(mt + 1) * NVS],
                        axis=mybir.AxisListType.X, op=ALU.add)
                nc.vector.tensor_reduce(
                    lse_sb[:, 3:4], se_parts[:, 3 * NVS:3 * NVS + 16],
                    axis=mybir.AxisListType.X, op=ALU.add)
                lse2 = state.tile([128, 4], F32)
                nc.scalar.activation(lse2[:], lse_sb[:], AF.Ln)
                nc.sync.dma_start(out_lse[:], lse2[:])
                nc.sync.dma_start(out_gd[:], gd_sb[:])

    nc.compile()
    return nc


def _perm_gates(w4h, s_ifo):
    """Rows (4H, ...) in torch order i,f,g,o -> [g, f*s, i*s, o*s]."""
    wi, wf, wg, wo = np.split(np.asarray(w4h), 4, axis=0)
    return np.concatenate([wg, wf * s_ifo, wi * s_ifo, wo * s_ifo], axis=0)


def _pack_lhsT(wt, kchs, mchs):
    """wt: (K, M) = W.T -> (128, kchs*mchs*128), col=(kch*mchs+mch)*128+m."""
    tiles = [wt[k * 128:(k + 1) * 128, m * 128:(m + 1) * 128]
             for k in range(kchs) for m in range(mchs)]
    return np.ascontiguousarray(np.concatenate(tiles, axis=1)).astype(bf16)


def _pack_dr(wt, fp8):
    """wt: (256, M) -> fp8 (128, (mch kch 128)): DoubleRow gate layout."""
    mchs = wt.shape[1] // 128
    tiles = [wt[k * 128:(k + 1) * 128, m * 128:(m + 1) * 128]
             for m in range(mchs) for k in range(2)]
    return np.ascontiguousarray(np.concatenate(tiles, axis=1)).astype(fp8)


def _pack_xT(x):
    """x: (rows, 256) -> (128, 2*rows), col = ech*rows + r."""
    a = np.ascontiguousarray(np.asarray(x).T)
    return np.ascontiguousarray(
        np.concatenate([a[:128], a[128:]], axis=1)).astype(bf16)


def _pack_bias(b):
    return np.ascontiguousarray(
        np.asarray(b).reshape(GCH, 128).T).astype(np.float32)


_NC_CACHE = {}
_RUN_KWARGS = {}      # test harness may set e.g. {"trace": True}
_LAST_RESULTS = None  # BassKernelResults of the most recent kernel() call
_LAST_INMAPS = None


def _get_program():
    if "nc" not in _NC_CACHE:
        _NC_CACHE["nc"] = build_program()
    return _NC_CACHE["nc"]


def _shared_inmap(enc_Wih_f, enc_Whh_f, enc_b_f, enc_Wih_b, enc_Whh_b,
                  enc_b_b, dec_Wih, dec_Whh, dec_b, Wh, Wc, Watt, Wcomb,
                  Wvocab):
    fp8 = mybir.dt.np(FP8)
    wvT = np.ascontiguousarray(np.asarray(Wvocab).T) * WSCL  # (256, V) * 64
    wv_packed = np.ascontiguousarray(
        np.concatenate([wvT[:128], wvT[128:]], axis=1)).astype(fp8)
    return {
        "wih_f": _pack_lhsT(_perm_gates(enc_Wih_f, 0.5).T, ECH, GCH),
        "wih_b": _pack_lhsT(_perm_gates(enc_Wih_b, 0.5).T, ECH, GCH),
        "whh_f": _pack_lhsT(_perm_gates(np.asarray(enc_Whh_f) * 0.5,
                                        0.5).T, HCH, GCH),
        "whh_b": _pack_lhsT(_perm_gates(np.asarray(enc_Whh_b) * 0.5,
                                        0.5).T, HCH, GCH),
        "benc_f": _pack_bias(_perm_gates(enc_b_f, 0.5)),
        "benc_b": _pack_bias(_perm_gates(enc_b_b, 0.5)),
        "wihe": _pack_lhsT(_perm_gates(np.asarray(dec_Wih)[:, :E],
                                       0.5).T, ECH, GCH),
        "wiho": _pack_lhsT(_perm_gates(np.asarray(dec_Wih)[:, E:],
                                       0.5).T, HCH, GCH),
        "whhd": _pack_lhsT(_perm_gates(np.asarray(dec_Whh) * 0.5,
                                       0.5).T, HCH, GCH),
        "bdec": _pack_bias(_perm_gates(dec_b, 0.5)),
        "wcomb_l": _pack_lhsT(np.asarray(Wcomb).T * 0.5, 6, 2),
        "wh_l": _pack_lhsT(np.asarray(Wh).T, 4, 2),
        "wc_l": _pack_lhsT(np.asarray(Wc).T, 4, 2),
        "watt_l": _pack_lhsT(np.asarray(Watt).T * (0.25 * ESCL), 4, 2),
        "wvt": wv_packed,
    }


def _core_inmap(shared, source_padded, target_padded, src_emb, tgt_emb,
                Wvocab, c):
    bs = slice(BL * c, BL * (c + 1))
    src = source_padded[:, bs]
    tgt = target_padded[:, bs]
    X = src_emb[src]                      # (S, 8, E)
    Y = tgt_emb[tgt[:-1]]                 # (TD, 8, E)
    wg = Wvocab[tgt[1:].reshape(-1)]      # (504, 256)
    m = dict(shared)
    m["xf_t"] = _pack_xT(X.reshape(S * BL, E))
    m["xb_t"] = _pack_xT(X[::-1].reshape(S * BL, E))
    m["yt"] = _pack_xT(Y.reshape(TD * BL, E))
    m["wgt"] = _pack_xT(wg)
    return m


def postprocess_core(lse, gd, target_padded, c):
    gd = np.asarray(gd).reshape(-1)
    lse_flat = np.asarray(lse).T.reshape(-1)[:NR]
    gold_logit = gd[:NR] + gd[512:512 + NR]
    p_gold = (gold_logit - lse_flat).reshape(TD, BL)
    mask = (np.asarray(target_padded)[1:, BL * c:BL * (c + 1)] != 0)
    return (p_gold * mask).sum(axis=0)


def build_inmap_core0(inputs):
    """For simcheck: build the program + core-0 input map."""
    nc = _get_program()
    shared = _shared_inmap(
        inputs["enc_Wih_f"], inputs["enc_Whh_f"], inputs["enc_b_f"],
        inputs["enc_Wih_b"], inputs["enc_Whh_b"], inputs["enc_b_b"],
        inputs["dec_Wih"], inputs["dec_Whh"], inputs["dec_b"],
        inputs["Wh"], inputs["Wc"], inputs["Watt"], inputs["Wcomb"],
        inputs["Wvocab"])
    m = _core_inmap(shared, np.asarray(inputs["source_padded"]),
                    np.asarray(inputs["target_padded"]),
                    np.asarray(inputs["src_emb"]),
                    np.asarray(inputs["tgt_emb"]),
                    np.asarray(inputs["Wvocab"]), 0)
    return nc, m


def kernel(source_padded, target_padded, src_emb, tgt_emb,
           enc_Wih_f, enc_Whh_f, enc_b_f, enc_Wih_b, enc_Whh_b, enc_b_b,
           dec_Wih, dec_Whh, dec_b, Wh, Wc, Watt, Wcomb, Wvocab):
    source_padded = np.asarray(source_padded)
    target_padded = np.asarray(target_padded)
    src_emb = np.asarray(src_emb)
    tgt_emb = np.asarray(tgt_emb)
    Wvocab = np.asarray(Wvocab)
    nc = _get_program()

    shared = _shared_inmap(enc_Wih_f, enc_Whh_f, enc_b_f, enc_Wih_b,
                           enc_Whh_b, enc_b_b, dec_Wih, dec_Whh, dec_b,
                           Wh, Wc, Watt, Wcomb, Wvocab)
    in_maps = [
        _core_inmap(shared, source_padded, target_padded, src_emb, tgt_emb,
                    Wvocab, c)
        for c in range(NCORES)]

    r = run_bass_kernel_spmd(nc, in_maps, list(range(NCORES)),
                             **_RUN_KWARGS)
    global _LAST_RESULTS, _LAST_INMAPS
    _LAST_RESULTS = r
    _LAST_INMAPS = in_maps

    out = np.zeros(B, np.float32)
    for c in range(NCORES):
        out[BL * c:BL * (c + 1)] = postprocess_core(
            r.results[c]["out_lse"], r.results[c]["out_gd"][0],
            target_padded, c)
    return out


# revision 18
# speedup vs baseline: 1.1945x; 1.1945x over previous
"""Trainium2 Bass kernel for the DPPNMT seq2seq LSTM+attention model (v5).

Sharding: data-parallel over batch (64 -> 8 per core, 8 cores), params
replicated. Each core runs encoder+decoder+vocab projection+logsumexp for
its 8 batch elements; host combines per-core gold/lse partials into (64,).

v5 (596.7us on HW, from the 889us v2 baseline):
- LSTM cell via fused scalar_tensor_tensor ops (u=(tanh(f,i)+1)*(c,g),
  h=(tanh(o)+1)*tanh(c)); single-source recurrent consumers halve the
  gate matmul count (encoder 33->17 per step/dir).
- z-tanh split per gate group (g,f,i as soon as their O-part matmuls
  land; o later, off the critical path) in both encoder and decoder.
- Softmax: exp writes straight into the block-diag alpha tile (two
  partition-half ACTs, no copies); Z broadcast to all 128 partitions by
  one all-ones matmul so 1/Z is a full-partition reciprocal and
  a_t = pat * rz is a single tensor_mul.
- Decoder PSUM split into four single-buffer bank tags (z | peB+Z | pat
  | po): tile-granular WAR/RAW tracking no longer serializes exp behind
  gate/Wcomb writes (+the emission order keeps gates(t+1) off exp's
  semaphore target).
- Attention score stationaries stored b-major (epv2) so their
  LDWEIGHTS are contiguous (2x faster issue than the strided layout).
- fp8e4 DoubleRow ONLY for the 512-col vocab-projection streams (K=256
  in one matmul; O rows get a per-step fp8 copy). Measured on HW:
  DoubleRow costs ~127ns/matmul vs ~27ns for small-N bf16 pairs, so the
  gate/score/Wcomb matmuls stay bf16.
- Vocab chunks interleaved into decoder steps t>=16 (4 x 512-col chunks
  + exps per step, placed in the ACT-idle windows of the serial cycle);
  encoder transposes double-buffered through the pep pool.
"""

from contextlib import ExitStack

import numpy as np
import ml_dtypes

import concourse.bass as bass
import concourse.tile as tile
from concourse import bacc, mybir
from concourse.bass_utils import run_bass_kernel_spmd
from concourse.masks import make_identity

BF16 = mybir.dt.bfloat16
F32 = mybir.dt.float32
FP8 = mybir.dt.float8e4
AF = mybir.ActivationFunctionType
ALU = mybir.AluOpType
DR = mybir.MatmulPerfMode.DoubleRow

S, T, B, E, H, V = 64, 64, 64, 256, 256, 32000
NCORES = 8
BL = B // NCORES          # local batch = 8
TD = T - 1                # decoder steps = 63
GCH = 8                   # gate chunks (4H/128)
ECH = 2
HCH = 2
NR = TD * BL              # 504 vocab rows per core
VBLK = 512                # vocab cols per chunk (1 psum bank)
NVS = 63                  # chunks per mt pass (62 full + 1 ragged 256)
WSCL = 64.0               # fp8 Wvocab scale
GSCL = 16.0               # fp8 gate/Wcomb weight scale
ESCL = 8.0                # encproj scale
bf16 = ml_dtypes.bfloat16

HST = (S + 1) * 8
OST = (TD + 1) * 8


def build_program():
    nc = bacc.Bacc("TRN2", target_bir_lowering=False, debug=False)

    def din(name, shape, dt=BF16):
        return nc.dram_tensor(name, shape, dt, kind="ExternalInput").ap()

    xf_t = din("xf_t", [128, ECH * S * BL])
    xb_t = din("xb_t", [128, ECH * S * BL])
    wih_f = din("wih_f", [128, ECH * GCH * 128])
    wih_b = din("wih_b", [128, ECH * GCH * 128])
    whh_f = din("whh_f", [128, HCH * GCH * 128])
    whh_b = din("whh_b", [128, HCH * GCH * 128])
    benc_f = din("benc_f", [128, GCH], F32)
    benc_b = din("benc_b", [128, GCH], F32)
    yt = din("yt", [128, ECH * TD * BL])
    wihe = din("wihe", [128, ECH * GCH * 128])
    wiho = din("wiho", [128, HCH * GCH * 128])
    whhd = din("whhd", [128, HCH * GCH * 128])
    bdec = din("bdec", [128, GCH], F32)
    wcomb_l = din("wcomb_l", [128, 6 * 2 * 128])
    wh_l = din("wh_l", [128, 4 * 2 * 128])
    wc_l = din("wc_l", [128, 4 * 2 * 128])
    watt_l = din("watt_l", [128, 4 * 2 * 128])
    wvt = din("wvt", [128, HCH * V], FP8)
    wgt = din("wgt", [128, HCH * NR])
    out_lse = nc.dram_tensor("out_lse", [128, 4], F32,
                             kind="ExternalOutput").ap()
    out_gd = nc.dram_tensor("out_gd", [1, 1024], F32,
                            kind="ExternalOutput").ap()

    with tile.TileContext(nc) as tc:
        with ExitStack() as ctx:
            consts = ctx.enter_context(tc.tile_pool(name="consts", bufs=1))
            wsb = ctx.enter_context(tc.tile_pool(name="wsb", bufs=1))
            state = ctx.enter_context(tc.tile_pool(name="state", bufs=1))

            id128 = consts.tile([128, 128], BF16)
            make_identity(nc, id128[:])
            ones_bf = consts.tile([128, 1], BF16)
            nc.vector.memset(ones_bf[:], 1.0)
            ones_all = consts.tile([128, 128], BF16)
            nc.vector.memset(ones_all[:], 1.0)

            def load(ap_dram, dt=BF16):
                t = wsb.tile(list(ap_dram.shape), dt,
                             tag=ap_dram.tensor.name + "_sb")
                nc.sync.dma_start(t[:], ap_dram[:])
                return t

            xf_sb, xb_sb = load(xf_t), load(xb_t)
            wihf_sb, wihb_sb = load(wih_f), load(wih_b)
            whhf_sb, whhb_sb = load(whh_f), load(whh_b)
            bencf_sb, bencb_sb = load(benc_f, F32), load(benc_b, F32)
            yt_sb = load(yt)
            wihe_sb = load(wihe)
            wiho_sb, whhd_sb = load(wiho), load(whhd)
            bdec_sb = load(bdec, F32)
            wcomb_sb = load(wcomb_l)
            wh_sb, wc_sb, watt_sb = load(wh_l), load(wc_l), load(watt_l)
            wgt_sb = load(wgt)

            # resident fp8 Wvocab^T, streamed in 8 DMA pieces
            wv_sb = state.tile([128, HCH * V], FP8)
            for p in range(8):
                w0 = p * (HCH * V // 8)
                w1 = (p + 1) * (HCH * V // 8)
                nc.sync.dma_start(wv_sb[:, w0:w1], wvt[:, w0:w1])

            # persistent state
            hf_all = state.tile([128, 2 * HST], BF16)
            hb_all = state.tile([128, 2 * HST], BF16)
            for hx in (hf_all, hb_all):
                nc.vector.memset(hx[:, 0:8], 0.0)
                nc.vector.memset(hx[:, HST:HST + 8], 0.0)
            # cell tiles: [c(0:16) | tg(16:32) | f i o]
            Wf = state.tile([128, 80], F32)
            Wb = state.tile([128, 80], F32)
            Wd = state.tile([128, 80], F32)
            nc.vector.memset(Wf[:, 0:16], 0.0)
            nc.vector.memset(Wb[:, 0:16], 0.0)
            outsT = state.tile([128, 2 * OST], BF16)
            nc.vector.memset(outsT[:, 0:8], 0.0)
            nc.vector.memset(outsT[:, OST:OST + 8], 0.0)
            outsT8 = state.tile([128, 2 * OST], FP8)
            hdec0 = state.tile([128, 16], BF16)
            zxf = state.tile([128, S * 64], BF16)
            zxb = state.tile([128, S * 64], BF16)
            zyb = state.tile([128, TD * 64], BF16)
            ehs_cs = state.tile([128, 16 * 128], BF16)
            encprojT = state.tile([128, HCH * BL * S], BF16)
            epv2 = state.tile([128, BL * HCH * S], BF16)
            ablk = state.tile([128, 8], BF16)
            nc.vector.memset(ablk[:], 0.0)
            se_parts = state.tile([128, 3 * NVS + 16], F32)
            nc.vector.memset(se_parts[:], 1.0)
            lse_sb = state.tile([128, 4], F32)
            gd_sb = state.tile([1, 1024], F32)
            nc.vector.memset(gd_sb[:], 0.0)
            tmp_gd = state.tile([128, 2 * NR], BF16)
            rz_sb = state.tile([128, 8], F32)

            with ExitStack() as rctx:
                pep = rctx.enter_context(
                    tc.tile_pool(name="pep", bufs=2, space="PSUM"))
                pz = rctx.enter_context(
                    tc.tile_pool(name="pz", bufs=2, space="PSUM"))
                psmall = rctx.enter_context(
                    tc.tile_pool(name="psmall", bufs=1, space="PSUM"))
                work = rctx.enter_context(tc.tile_pool(name="work", bufs=2))

                # ---- zx = x @ Wih^T + b; zyb phase deferred past the
                # encoder so its Act work fills encoder idle ----
                def zx_phase(x_sb, wih_sb, b_sb, zx, nt):
                    zxv = zx[:].rearrange("p (t g b) -> p t g b",
                                          g=GCH, b=BL)
                    for gch in range(GCH):
                        ps = pep.tile([128, S * BL], F32, tag="pep",
                                      name="ps")
                        for ech in range(ECH):
                            nc.tensor.matmul(
                                ps[:, 0:nt * BL],
                                wih_sb[:, (ech * GCH + gch) * 128:
                                       (ech * GCH + gch + 1) * 128],
                                x_sb[:, ech * nt * BL:(ech + 1) * nt * BL],
                                start=(ech == 0), stop=(ech == ECH - 1))
                        nc.scalar.activation(
                            zxv[:, 0:nt, gch, :], ps[:, 0:nt * BL],
                            AF.Identity, bias=b_sb[:, gch:gch + 1])

                zx_phase(xf_sb, wihf_sb, bencf_sb, zxf, S)
                zx_phase(xb_sb, wihb_sb, bencb_sb, zxb, S)

                # ---- encoder: two dir chains, op-interleaved emission ----
                hfv = hf_all[:].rearrange("p (c t b) -> p c t b", c=2, b=BL)
                hbv = hb_all[:].rearrange("p (c t b) -> p c t b", c=2, b=BL)

                enc_prev = {}

                def enc_step(Wt, h_all, hv, whh_sb, zx, pfx, t):
                    z = pz.tile([128, 64], F32, tag="z" + pfx, name="z")
                    pt = enc_prev.get(pfx)
                    nc.tensor.matmul(z[:], id128[:],
                                     zx[:, t * 64:(t + 1) * 64],
                                     start=True, stop=(pt is None),
                                     skip_group_check=True)
                    if pt is not None:
                        for gch in range(6):
                            for kch in range(HCH):
                                nc.tensor.matmul(
                                    z[:, gch * 8:(gch + 1) * 8],
                                    whh_sb[:, (kch * GCH + gch) * 128:
                                           (kch * GCH + gch + 1) * 128],
                                    h_all[:, kch * HST + pt * 8:
                                          kch * HST + pt * 8 + 8],
                                    start=False, stop=False,
                                    skip_group_check=True)
                    # g,f,i tanh as soon as their gate chunks are done
                    nc.scalar.activation(Wt[:, 16:64], z[:, 0:48], AF.Tanh)
                    if pt is not None:
                        for gch in (6, 7):
                            for kch in range(HCH):
                                nc.tensor.matmul(
                                    z[:, gch * 8:(gch + 1) * 8],
                                    whh_sb[:, (kch * GCH + gch) * 128:
                                           (kch * GCH + gch + 1) * 128],
                                    h_all[:, kch * HST + pt * 8:
                                          kch * HST + pt * 8 + 8],
                                    start=False,
                                    stop=(gch == GCH - 1 and kch == HCH - 1),
                                    skip_group_check=True)
                    yield
                    nc.scalar.activation(Wt[:, 64:80], z[:, 48:64], AF.Tanh)
                    yield
                    # u = (tanh(f,i)+1) * (c, tg)
                    u = work.tile([128, 32], F32, tag=pfx + "u", name="u")
                    nc.vector.scalar_tensor_tensor(
                        u[:], Wt[:, 32:64], 1.0, Wt[:, 0:32],
                        ALU.add, ALU.mult)
                    yield
                    C2 = work.tile([128, 16], F32, tag=pfx + "C2", name="C2")
                    nc.vector.tensor_add(C2[:], u[:, 0:16], u[:, 16:32])
                    yield
                    tc_ = work.tile([128, 16], BF16, tag=pfx + "tc",
                                    name="tc")
                    nc.scalar.activation(tc_[:], C2[:], AF.Tanh, scale=0.5)
                    yield
                    # h = (tanh(o)+1) * tanh(c')  (single source for gates)
                    ov = Wt[:, 64:80].rearrange("p (c b) -> p c b", c=2)
                    tv = tc_[:].rearrange("p (c b) -> p c b", c=2)
                    nc.vector.scalar_tensor_tensor(
                        hv[:, :, t + 1, :], ov, 1.0, tv,
                        ALU.add, ALU.mult)
                    enc_prev[pfx] = t + 1
                    yield
                    nc.gpsimd.tensor_scalar_mul(Wt[:, 0:16], C2[:], 0.5)

                def drive2(*gens):
                    gens = [gn for gn in gens if gn is not None]
                    while gens:
                        keep = []
                        for gn in gens:
                            try:
                                next(gn)
                                keep.append(gn)
                            except StopIteration:
                                pass
                        gens = keep

                for t in range(S):
                    drive2(
                        enc_step(Wf, hf_all, hfv, whhf_sb, zxf, "f", t),
                        enc_step(Wb, hb_all, hbv, whhb_sb, zxb, "b", t))
                    if t == 2:
                        zx_phase(yt_sb, wihe_sb, bdec_sb, zyb, TD)

                # ---- ehs_cs[64u+s, (pair,mt)] via PE transposes ----
                for pair in range(4):
                    for mt in range(4):
                        srcv = hfv if mt < 2 else hbv
                        pt = pep.tile([128, 128], BF16, tag="pep",
                                      name="pt")
                        for u in range(2):
                            in_ap = srcv[:, mt % 2, 1:S + 1, 2 * pair + u]
                            nc.tensor.transpose(pt[u * 64:(u + 1) * 64, :],
                                                in_ap, id128[:])
                        nc.vector.tensor_copy(
                            ehs_cs[:, (pair * 4 + mt) * 128:
                                   (pair * 4 + mt + 1) * 128], pt[:])

                # ---- encproj^T = (2*Watt) @ ehs^T  (x8 scale for fp8) ----
                for mch in range(HCH):
                    ps = pep.tile([128, S * BL], F32, tag="pep")
                    for kch in range(4):
                        srch = hf_all if kch < 2 else hb_all
                        rhs = srch[:, (kch % 2) * HST + 8:
                                   (kch % 2) * HST + HST]
                        nc.tensor.matmul(
                            ps[:],
                            watt_sb[:, (kch * 2 + mch) * 128:
                                    (kch * 2 + mch + 1) * 128],
                            rhs, start=(kch == 0), stop=(kch == 3))
                    nc.scalar.activation(
                        encprojT[:, mch * BL * S:(mch + 1) * BL * S],
                        ps[:], AF.Copy)
                # b-major copy: per-b score stationaries become contiguous
                e_in = encprojT[:].rearrange("p (c s b) -> p b c s",
                                             c=2, b=BL)
                e_out = epv2[:].rearrange("p (b c s) -> p b c s",
                                          b=BL, c=2)
                nc.vector.tensor_copy(e_out, e_in)

                # ---- decoder init: dec_h/dec_c projections ----
                cfb = work.tile([128, 16], BF16, tag="cfb")
                cbb = work.tile([128, 16], BF16, tag="cbb")
                nc.vector.tensor_copy(cfb[:], Wf[:, 0:16])
                nc.vector.tensor_copy(cbb[:], Wb[:, 0:16])
                pinit = psmall.tile([128, 32], F32, tag="po")
                for (w_sb, off, hsrc, csrc) in (
                        (wh_sb, 0, (hf_all, hb_all), None),
                        (wc_sb, 16, None, (cfb, cbb))):
                    for mch in range(HCH):
                        for kch in range(4):
                            if hsrc is not None:
                                hx = hsrc[0] if kch < 2 else hsrc[1]
                                rhs = hx[:, (kch % 2) * HST + S * 8:
                                         (kch % 2) * HST + S * 8 + 8]
                            else:
                                cx = csrc[0] if kch < 2 else csrc[1]
                                rhs = cx[:, (kch % 2) * 8:(kch % 2) * 8 + 8]
                            nc.tensor.matmul(
                                pinit[:, off + mch * 8:off + (mch + 1) * 8],
                                w_sb[:, (kch * 2 + mch) * 128:
                                     (kch * 2 + mch + 1) * 128],
                                rhs, start=(kch == 0), stop=(kch == 3))
                nc.vector.tensor_copy(hdec0[:], pinit[:, 0:16])
                nc.vector.tensor_copy(Wd[:, 0:16], pinit[:, 16:32])

            # ---- decoder with interleaved vocab chunks ----
            with ExitStack() as dctx:
                pv = dctx.enter_context(
                    tc.tile_pool(name="pv", bufs=4, space="PSUM"))
                pd = dctx.enter_context(
                    tc.tile_pool(name="pd", bufs=1, space="PSUM"))
                workd = dctx.enter_context(tc.tile_pool(name="workd", bufs=2))

                ovv = outsT[:].rearrange("p (c t b) -> p c t b", c=2, b=BL)
                ov8v = outsT8[:].rearrange("p (c t b) -> p c t b", c=2, b=BL)
                ovdr = outsT8[:].rearrange("p (c r) -> p c r", c=2)
                wvdr = wv_sb[:].rearrange("p (c n) -> p c n", c=2)

                # single-buffer PSUM bank tags (decoupled dep tracking)
                z_ps = pd.tile([128, 64], F32, tag="dz")
                sc_ps = pd.tile([128, 12], F32, tag="dsc")
                pat_ps = pd.tile([128, 32], F32, tag="dpat")
                po_ps = pd.tile([128, 16], F32, tag="dpo")
                peB = sc_ps[:, 0:4]
                Zb = sc_ps[:, 4:12]

                chunk_queue = []

                def vocab_mm(mt, vs):
                    w = VBLK if vs < NVS - 1 else (V - (NVS - 1) * VBLK)
                    m = 128 if mt < 3 else 120
                    ntau = 16 if mt < 3 else 15
                    col0 = vs * VBLK
                    pvt = pv.tile([128, VBLK], F32, tag="pv")
                    nc.tensor.matmul(
                        pvt[0:m, 0:w],
                        ovdr[:, :, (mt * 16 + 1) * 8:
                             (mt * 16 + 1 + ntau) * 8],
                        wvdr[:, :, col0:col0 + w],
                        start=True, stop=True, perf_mode=DR)
                    return (mt, vs, pvt, w, m)

                def vocab_exp(rec):
                    mt, vs, pvt, w, m = rec
                    nc.scalar.activation(
                        pvt[0:m, 0:w], pvt[0:m, 0:w], AF.Exp,
                        scale=1.0 / WSCL,
                        accum_out=se_parts[0:m, mt * NVS + vs:
                                           mt * NVS + vs + 1])

                def gates_h(z, h_bf, t_next):
                    """inject + H-part for step t_next."""
                    nc.tensor.matmul(z, id128[:],
                                     zyb[:, t_next * 64:(t_next + 1) * 64],
                                     start=True, stop=False,
                                     skip_group_check=True)
                    for gch in range(GCH):
                        for kch in range(HCH):
                            nc.tensor.matmul(
                                z[:, gch * 8:(gch + 1) * 8],
                                whhd_sb[:, (kch * GCH + gch) * 128:
                                        (kch * GCH + gch + 1) * 128],
                                h_bf[:, kch * 8:(kch + 1) * 8],
                                start=False, stop=False,
                                skip_group_check=True)

                def gates_o(z, t_next, g0, g1):
                    for gch in range(g0, g1):
                        for kch in range(HCH):
                            nc.tensor.matmul(
                                z[:, gch * 8:(gch + 1) * 8],
                                wiho_sb[:, (kch * GCH + gch) * 128:
                                        (kch * GCH + gch + 1) * 128],
                                outsT[:, kch * OST + t_next * 8:
                                      kch * OST + t_next * 8 + 8],
                                start=False,
                                stop=(gch == GCH - 1 and kch == HCH - 1),
                                skip_group_check=True)

                abv = ablk[:].rearrange("p (j u) -> p j u", u=2)

                def tanh1():
                    # g,f,i gate chunks -> Wd[16:64]
                    nc.scalar.activation(Wd[:, 16:64], z_ps[:, 0:48],
                                         AF.Tanh)

                gates_h(z_ps[:], hdec0, 0)
                gates_o(z_ps[:], 0, 0, 6)
                tanh1()
                gates_o(z_ps[:], 0, 6, 8)

                for t in range(TD):
                    if t == 16:
                        chunk_queue.extend((0, vs) for vs in range(NVS))
                    elif t == 32:
                        chunk_queue.extend((1, vs) for vs in range(NVS))
                    elif t == 48:
                        chunk_queue.extend((2, vs) for vs in range(NVS))
                    exps = []
                    if t >= 16:
                        for _ in range(4):
                            if chunk_queue:
                                mt_, vs_ = chunk_queue.pop(0)
                                exps.append(vocab_mm(mt_, vs_))

                    # -- cell (tanh1 of this step ran at end of t-1) --
                    nc.scalar.activation(Wd[:, 64:80], z_ps[:, 48:64],
                                         AF.Tanh)
                    u = workd.tile([128, 32], F32, tag="u")
                    nc.vector.scalar_tensor_tensor(
                        u[:], Wd[:, 32:64], 1.0, Wd[:, 0:32],
                        ALU.add, ALU.mult)
                    C2 = workd.tile([128, 16], F32, tag="C2")
                    nc.vector.tensor_add(C2[:], u[:, 0:16], u[:, 16:32])
                    tc_ = workd.tile([128, 16], BF16, tag="tc")
                    nc.scalar.activation(tc_[:], C2[:], AF.Tanh, scale=0.5)
                    nc.gpsimd.tensor_scalar_mul(Wd[:, 0:16], C2[:], 0.5)
                    hdec = workd.tile([128, 16], BF16, tag="hdec")
                    nc.vector.scalar_tensor_tensor(
                        hdec[:], Wd[:, 64:80], 1.0, tc_[:],
                        ALU.add, ALU.mult)
                    if exps:
                        vocab_exp(exps.pop(0))
                    # -- scores: e = ep . h (contiguous per-b stationaries) --
                    for b in range(BL):
                        u_, j_ = b % 2, b // 2
                        for ch in range(HCH):
                            nc.tensor.matmul(
                                peB[u_ * 64:u_ * 64 + 64, j_:j_ + 1],
                                epv2[:, b * 128 + ch * 64:
                                     b * 128 + ch * 64 + 64],
                                hdec[:, ch * 8 + b:ch * 8 + b + 1],
                                start=(ch == 0), stop=(ch == 1),
                                skip_group_check=True)
                    # -- gates t+1 H-part + Wcomb h-part: fill exp window --
                    if t + 1 < TD:
                        gates_h(z_ps[:], hdec, t + 1)
                    for kch in (4, 5):
                        for mch in range(HCH):
                            nc.tensor.matmul(
                                po_ps[:, mch * 8:(mch + 1) * 8],
                                wcomb_sb[:, (kch * 2 + mch) * 128:
                                         (kch * 2 + mch + 1) * 128],
                                hdec[:, (kch - 4) * 8:(kch - 3) * 8],
                                start=(kch == 4), stop=False,
                                skip_group_check=True)
                    # -- exp directly into block-diag alpha halves --
                    nc.scalar.activation(abv[0:64, :, 0], peB[0:64, :],
                                         AF.Exp, scale=1.0 / ESCL)
                    nc.scalar.activation(abv[64:128, :, 1], peB[64:128, :],
                                         AF.Exp, scale=1.0 / ESCL)
                    # -- Z broadcast to all partitions: one ones-matmul --
                    nc.tensor.matmul(Zb, ones_all[:], ablk[:, 0:8],
                                     start=True, stop=True,
                                     skip_group_check=True)
                    nc.vector.reciprocal(rz_sb[:], Zb)
                    # -- context (block-diag alpha x ehs) --
                    for pair in range(4):
                        for mt in range(4):
                            nc.tensor.matmul(
                                pat_ps[:, mt * 8 + pair * 2:
                                       mt * 8 + pair * 2 + 2],
                                ehs_cs[:, (pair * 4 + mt) * 128:
                                       (pair * 4 + mt + 1) * 128],
                                ablk[:, 2 * pair:2 * pair + 2],
                                start=True, stop=True,
                                skip_group_check=True)
                    if exps:
                        vocab_exp(exps.pop(0))
                    if exps:
                        vocab_exp(exps.pop(0))
                    # -- a_t = pat * (1/Z) --
                    aTb = workd.tile([128, 32], BF16, tag="aTb")
                    pat_v = pat_ps[:].rearrange("p (m b) -> p m b", b=8)
                    rz_v = rz_sb[:].rearrange("p (m b) -> p m b", m=1)
                    pat_b, rz_b = bass.broadcast_tensor_aps(pat_v, rz_v)
                    aTb_v = aTb[:].rearrange("p (m b) -> p m b", b=8)
                    nc.vector.tensor_mul(aTb_v, pat_b, rz_b)
                    # -- Wcomb a-part + output tanh --
                    for kch in range(4):
                        for mch in range(HCH):
                            nc.tensor.matmul(
                                po_ps[:, mch * 8:(mch + 1) * 8],
                                wcomb_sb[:, (kch * 2 + mch) * 128:
                                         (kch * 2 + mch + 1) * 128],
                                aTb[:, kch * 8:(kch + 1) * 8],
                                start=False,
                                stop=(kch == 3 and mch == HCH - 1),
                                skip_group_check=True)
                    nc.scalar.activation(ovv[:, :, t + 1, :],
                                         po_ps[:], AF.Tanh)
                    # fp8 copy of O row for the DoubleRow vocab matmuls
                    nc.vector.tensor_copy(ov8v[:, :, t + 1, :],
                                          ovv[:, :, t + 1, :])
                    if exps:
                        vocab_exp(exps.pop(0))
                    if t + 1 < TD:
                        gates_o(z_ps[:], t + 1, 0, 6)
                        tanh1()
                        gates_o(z_ps[:], t + 1, 6, 8)
                    while exps:
                        vocab_exp(exps.pop(0))

                # ---- non-mt3 leftovers (small chunks) ----
                mm_done = []
                for (mt, vs) in chunk_queue:
                    rec = vocab_mm(mt, vs)
                    mm_done.append(rec)
                    if len(mm_done) > 1:
                        vocab_exp(mm_done.pop(0))
                for rec in mm_done:
                    vocab_exp(rec)

            # ---- mt=3 vocab tail with wide chunks, then gold + lse ----
            with ExitStack() as tctx:
                pvb = tctx.enter_context(
                    tc.tile_pool(name="pvb", bufs=2, space="PSUM"))
                VB2 = 2048
                ovt = outsT8[:].rearrange("p (c r) -> p c r", c=2)
                wvt2 = wv_sb[:].rearrange("p (c n) -> p c n", c=2)
                tail_recs = []
                for vs in range(16):
                    w = VB2 if vs < 15 else (V - 15 * VB2)
                    col0 = vs * VB2
                    pvt = pvb.tile([128, VB2], F32, tag="pvb")
                    for n0 in range(0, w, 512):
                        nw = min(512, w - n0)
                        nc.tensor.matmul(
                            pvt[0:120, n0:n0 + nw],
                            ovt[:, :, 49 * 8:64 * 8],
                            wvt2[:, :, col0 + n0:col0 + n0 + nw],
                            start=True, stop=True, perf_mode=DR)
                    tail_recs.append((vs, pvt, w))
                    if len(tail_recs) > 1:
                        vs_, pvt_, w_ = tail_recs.pop(0)
                        nc.scalar.activation(
                            pvt_[0:120, 0:w_], pvt_[0:120, 0:w_], AF.Exp,
                            scale=1.0 / WSCL,
                            accum_out=se_parts[0:120, 3 * NVS + vs_:
                                               3 * NVS + vs_ + 1])
                for (vs_, pvt_, w_) in tail_recs:
                    nc.scalar.activation(
                        pvt_[0:120, 0:w_], pvt_[0:120, 0:w_], AF.Exp,
                        scale=1.0 / WSCL,
                        accum_out=se_parts[0:120, 3 * NVS + vs_:
                                           3 * NVS + vs_ + 1])

                # gold logits: dot(O_t, Wvocab[gold]) via ones-matmul
                ovf = outsT[:].rearrange("p (c t b) -> p c t b", c=2, b=BL)
                ov = ovf[:, :, 1:, :]
                wgv = wgt_sb[:].rearrange("p (c t b) -> p c t b", c=2, b=BL)
                tgv = tmp_gd[:].rearrange("p (c t b) -> p c t b", c=2, b=BL)
                nc.vector.tensor_mul(tgv, ov, wgv)
                pgd = pvb.tile([128, VB2], F32, tag="pvb", name="pgd")
                nc.tensor.matmul(pgd[0:1, 0:NR], ones_bf[:],
                                 tmp_gd[:, 0:NR], start=True, stop=True)
                nc.tensor.matmul(pgd[0:1, 512:512 + NR], ones_bf[:],
                                 tmp_gd[:, NR:2 * NR], start=True, stop=True)
                nc.scalar.activation(gd_sb[:, 0:NR], pgd[0:1, 0:NR], AF.Copy)
                nc.scalar.activation(gd_sb[:, 512:512 + NR],
                                     pgd[0:1, 512:512 + NR], AF.Copy)

                # lse reduce + ln
                for mt in range(3):
                    nc.vector.tensor_reduce(
                        lse_sb[:, mt:mt + 1],
                        se_parts[:, mt * NVS:(mt + 1) * NVS],
                        axis=mybir.AxisListType.X, op=ALU.add)
                nc.vector.tensor_reduce(
                    lse_sb[:, 3:4], se_parts[:, 3 * NVS:3 * NVS + 16],
                    axis=mybir.AxisListType.X, op=ALU.add)
                lse2 = state.tile([128, 4], F32)
                nc.scalar.activation(lse2[:], lse_sb[:], AF.Ln)
                nc.sync.dma_start(out_lse[:], lse2[:])
                nc.sync.dma_start(out_gd[:], gd_sb[:])

    nc.compile()
    return nc


def _perm_gates(w4h, s_ifo):
    """Rows (4H, ...) in torch order i,f,g,o -> [g, f*s, i*s, o*s]."""
    wi, wf, wg, wo = np.split(np.asarray(w4h), 4, axis=0)
    return np.concatenate([wg, wf * s_ifo, wi * s_ifo, wo * s_ifo], axis=0)


def _pack_lhsT(wt, kchs, mchs):
    """wt: (K, M) = W.T -> (128, kchs*mchs*128), col=(kch*mchs+mch)*128+m."""
    tiles = [wt[k * 128:(k + 1) * 128, m * 128:(m + 1) * 128]
             for k in range(kchs) for m in range(mchs)]
    return np.ascontiguousarray(np.concatenate(tiles, axis=1)).astype(bf16)


def _pack_dr(wt, fp8):
    """wt: (256, M) -> fp8 (128, (mch kch 128)): DoubleRow gate layout."""
    mchs = wt.shape[1] // 128
    tiles = [wt[k * 128:(k + 1) * 128, m * 128:(m + 1) * 128]
             for m in range(mchs) for k in range(2)]
    return np.ascontiguousarray(np.concatenate(tiles, axis=1)).astype(fp8)


def _pack_xT(x):
    """x: (rows, 256) -> (128, 2*rows), col = ech*rows + r."""
    a = np.ascontiguousarray(np.asarray(x).T)
    return np.ascontiguousarray(
        np.concatenate([a[:128], a[128:]], axis=1)).astype(bf16)


def _pack_bias(b):
    return np.ascontiguousarray(
        np.asarray(b).reshape(GCH, 128).T).astype(np.float32)


_NC_CACHE = {}
_RUN_KWARGS = {}      # test harness may set e.g. {"trace": True}
_LAST_RESULTS = None  # BassKernelResults of the most recent kernel() call
_LAST_INMAPS = None


def _get_program():
    if "nc" not in _NC_CACHE:
        _NC_CACHE["nc"] = build_program()
    return _NC_CACHE["nc"]


def _shared_inmap(enc_Wih_f, enc_Whh_f, enc_b_f, enc_Wih_b, enc_Whh_b,
                  enc_b_b, dec_Wih, dec_Whh, dec_b, Wh, Wc, Watt, Wcomb,
                  Wvocab):
    fp8 = mybir.dt.np(FP8)
    wvT = np.ascontiguousarray(np.asarray(Wvocab).T) * WSCL  # (256, V) * 64
    wv_packed = np.ascontiguousarray(
        np.concatenate([wvT[:128], wvT[128:]], axis=1)).astype(fp8)
    return {
        "wih_f": _pack_lhsT(_perm_gates(enc_Wih_f, 0.5).T, ECH, GCH),
        "wih_b": _pack_lhsT(_perm_gates(enc_Wih_b, 0.5).T, ECH, GCH),
        "whh_f": _pack_lhsT(_perm_gates(np.asarray(enc_Whh_f) * 0.5,
                                        0.5).T, HCH, GCH),
        "whh_b": _pack_lhsT(_perm_gates(np.asarray(enc_Whh_b) * 0.5,
                                        0.5).T, HCH, GCH),
        "benc_f": _pack_bias(_perm_gates(enc_b_f, 0.5)),
        "benc_b": _pack_bias(_perm_gates(enc_b_b, 0.5)),
        "wihe": _pack_lhsT(_perm_gates(np.asarray(dec_Wih)[:, :E],
                                       0.5).T, ECH, GCH),
        "wiho": _pack_lhsT(_perm_gates(np.asarray(dec_Wih)[:, E:],
                                       0.5).T, HCH, GCH),
        "whhd": _pack_lhsT(_perm_gates(np.asarray(dec_Whh) * 0.5,
                                       0.5).T, HCH, GCH),
        "bdec": _pack_bias(_perm_gates(dec_b, 0.5)),
        "wcomb_l": _pack_lhsT(np.asarray(Wcomb).T * 0.5, 6, 2),
        "wh_l": _pack_lhsT(np.asarray(Wh).T, 4, 2),
        "wc_l": _pack_lhsT(np.asarray(Wc).T, 4, 2),
        "watt_l": _pack_lhsT(np.asarray(Watt).T * (0.25 * ESCL), 4, 2),
        "wvt": wv_packed,
    }


def _core_inmap(shared, source_padded, target_padded, src_emb, tgt_emb,
                Wvocab, c):
    bs = slice(BL * c, BL * (c + 1))
    src = source_padded[:, bs]
    tgt = target_padded[:, bs]
    X = src_emb[src]                      # (S, 8, E)
    Y = tgt_emb[tgt[:-1]]                 # (TD, 8, E)
    wg = Wvocab[tgt[1:].reshape(-1)]      # (504, 256)
    m = dict(shared)
    m["xf_t"] = _pack_xT(X.reshape(S * BL, E))
    m["xb_t"] = _pack_xT(X[::-1].reshape(S * BL, E))
    m["yt"] = _pack_xT(Y.reshape(TD * BL, E))
    m["wgt"] = _pack_xT(wg)
    return m


def postprocess_core(lse, gd, target_padded, c):
    gd = np.asarray(gd).reshape(-1)
    lse_flat = np.asarray(lse).T.reshape(-1)[:NR]
    gold_logit = gd[:NR] + gd[512:512 + NR]
    p_gold = (gold_logit - lse_flat).reshape(TD, BL)
    mask = (np.asarray(target_padded)[1:, BL * c:BL * (c + 1)] != 0)
    return (p_gold * mask).sum(axis=0)


def build_inmap_core0(inputs):
    """For simcheck: build the program + core-0 input map."""
    nc = _get_program()
    shared = _shared_inmap(
        inputs["enc_Wih_f"], inputs["enc_Whh_f"], inputs["enc_b_f"],
        inputs["enc_Wih_b"], inputs["enc_Whh_b"], inputs["enc_b_b"],
        inputs["dec_Wih"], inputs["dec_Whh"], inputs["dec_b"],
        inputs["Wh"], inputs["Wc"], inputs["Watt"], inputs["Wcomb"],
        inputs["Wvocab"])
    m = _core_inmap(shared, np.asarray(inputs["source_padded"]),
                    np.asarray(inputs["target_padded"]),
                    np.asarray(inputs["src_emb"]),
                    np.asarray(inputs["tgt_emb"]),
                    np.asarray(inputs["Wvocab"]), 0)
    return nc, m


def kernel(source_padded, target_padded, src_emb, tgt_emb,
           enc_Wih_f, enc_Whh_f, enc_b_f, enc_Wih_b, enc_Whh_b, enc_b_b,
           dec_Wih, dec_Whh, dec_b, Wh, Wc, Watt, Wcomb, Wvocab):
    source_padded = np.asarray(source_padded)
    target_padded = np.asarray(target_padded)
    src_emb = np.asarray(src_emb)
    tgt_emb = np.asarray(tgt_emb)
    Wvocab = np.asarray(Wvocab)
    nc = _get_program()

    shared = _shared_inmap(enc_Wih_f, enc_Whh_f, enc_b_f, enc_Wih_b,
                           enc_Whh_b, enc_b_b, dec_Wih, dec_Whh, dec_b,
                           Wh, Wc, Watt, Wcomb, Wvocab)
    in_maps = [
        _core_inmap(shared, source_padded, target_padded, src_emb, tgt_emb,
                    Wvocab, c)
        for c in range(NCORES)]

    r = run_bass_kernel_spmd(nc, in_maps, list(range(NCORES)),
                             **_RUN_KWARGS)
    global _LAST_RESULTS, _LAST_INMAPS
    _LAST_RESULTS = r
    _LAST_INMAPS = in_maps

    out = np.zeros(B, np.float32)
    for c in range(NCORES):
        out[BL * c:BL * (c + 1)] = postprocess_core(
            r.results[c]["out_lse"], r.results[c]["out_gd"][0],
            target_padded, c)
    return out
